# revision 1
# baseline (speedup 1.0000x reference)
"""Devign GGNN model on 8 Trainium2 NeuronCores (Bass/Tile).

Strategy (data-parallel over dst-node shards):
  - 8 cores, core c owns nodes [c*4096, (c+1)*4096).
  - Node state kept transposed on-chip: hT [256 feat, 4096 nodes] bf16.
  - Per GGNN step:
      1. each core computes its shard of the per-etype transform
         H_all[n, t, :] = h[n] @ W_msg[t]  (row-major, bf16) and DMAs it
         to a DRAM bounce buffer,
      2. AllGather -> full table H_all [32768*4, 256] bf16 on every core,
      3. edges (sorted by dst, bucketed into 128-dst windows, chunks of
         128 edge slots) are processed: indirect-DMA row gather of
         H_all[src*4+etype], then a one-hot scatter matmul
         aT[w] += msg_chunk^T @ S_chunk accumulating in PSUM,
      4. GRU cell evaluated in transposed layout (PE matmuls for the
         six gate mats, ACT sigmoid/tanh, DVE elementwise).
  - Readout (conv1d/maxpool stacks + gated sum) is evaluated per graph
    with the conv expressed as tap-shifted matmuls.

All index/one-hot/weight-layout preprocessing is done on the host at
kernel() time and baked into the compiled program + per-core inputs.
"""
import sys
import numpy as np

for _p in ("/opt/trn_rl_repo",):
    if _p not in sys.path:
        sys.path.insert(0, _p)

import ml_dtypes

import concourse.bass as bass
import concourse.mybir as mybir
import concourse.tile as tile
from concourse import bacc
from concourse.bass_utils import run_bass_kernel_spmd

BF16 = ml_dtypes.bfloat16
F32 = np.float32

NCORES = 8
NN = 32768          # total nodes
IN_DIM = 128
OUT = 256
NT = 4              # edge types
NSTEPS = 8
NGRAPH = 128
NPC = NN // NCORES  # nodes per core = 4096
WIN = 128           # dst window size
NWIN = NPC // WIN   # 32 windows per core
GPC = NGRAPH // NCORES  # graphs per core = 16
LG = 256            # nodes per graph
CONCAT = IN_DIM + OUT  # 384

bf = mybir.dt.bfloat16
f32 = mybir.dt.float32
i32 = mybir.dt.int32
AF = mybir.ActivationFunctionType
ALU = mybir.AluOpType


# ---------------------------------------------------------------------------
# weight/bias image layout (shared between host packer and device slicer)
# ---------------------------------------------------------------------------
class WLayout:
    def __init__(self):
        self.col = 0
        self.off = {}

    def alloc(self, name, width):
        self.off[name] = self.col
        self.col += width
        return self.off[name]


def _make_wlayout():
    wl = WLayout()
    for t in range(NT):
        for kc in range(2):
            wl.alloc(f"wmsg_{t}_{kc}", OUT)        # rhs blocks [128, 256]
    for g in range(6):
        for kc in range(2):
            wl.alloc(f"wi_{g}_{kc}", 128)          # lhsT blocks
            wl.alloc(f"wh_{g}_{kc}", 128)
    for tap in range(3):
        for kc in range(2):
            for mo in range(2):
                wl.alloc(f"c1_{tap}_{kc}_{mo}", 128)
    for kc in range(2):
        for mo in range(2):
            wl.alloc(f"c2_{kc}_{mo}", 128)
    for tap in range(3):
        for kc in range(3):
            for mo in range(3):
                wl.alloc(f"cc1_{tap}_{kc}_{mo}", 128)
    for kc in range(3):
        for mo in range(3):
            wl.alloc(f"cc2_{kc}_{mo}", 128)
    for kc in range(2):
        wl.alloc(f"wy_{kc}", 1)
    for kc in range(3):
        wl.alloc(f"wz_{kc}", 1)
    return wl


def _make_blayout():
    bl = WLayout()
    for name in ("br", "bz", "big", "bhg"):
        bl.alloc(name, 2)       # [256] as two [128] cols
    bl.alloc("c1b", 2)
    bl.alloc("c2b", 2)
    bl.alloc("cc1b", 3)
    bl.alloc("cc2b", 3)
    bl.alloc("by", 1)
    bl.alloc("bz_", 1)
    return bl


def _pack_weights(wl, W_msg, gru_Wi, gru_Wh, conv1_w, conv2_w, convc1_w, convc2_w, wy, wz):
    img = np.zeros((128, wl.col), np.float32)

    def put(name, block):
        o = wl.off[name]
        img[:, o:o + block.shape[1]] = block

    for t in range(NT):
        for kc in range(2):
            put(f"wmsg_{t}_{kc}", W_msg[t][kc * 128:(kc + 1) * 128, :])
    for g in range(6):
        for kc in range(2):
            put(f"wi_{g}_{kc}", gru_Wi[kc * 128:(kc + 1) * 128, g * 128:(g + 1) * 128])
            put(f"wh_{g}_{kc}", gru_Wh[kc * 128:(kc + 1) * 128, g * 128:(g + 1) * 128])
    for tap in range(3):
        w_t = conv1_w[:, :, tap].T  # [i, o]
        for kc in range(2):
            for mo in range(2):
                put(f"c1_{tap}_{kc}_{mo}", w_t[kc * 128:(kc + 1) * 128, mo * 128:(mo + 1) * 128])
    w2 = conv2_w[:, :, 0].T
    for kc in range(2):
        for mo in range(2):
            put(f"c2_{kc}_{mo}", w2[kc * 128:(kc + 1) * 128, mo * 128:(mo + 1) * 128])
    for tap in range(3):
        w_t = convc1_w[:, :, tap].T
        for kc in range(3):
            for mo in range(3):
                put(f"cc1_{tap}_{kc}_{mo}", w_t[kc * 128:(kc + 1) * 128, mo * 128:(mo + 1) * 128])
    wc2 = convc2_w[:, :, 0].T
    for kc in range(3):
        for mo in range(3):
            put(f"cc2_{kc}_{mo}", wc2[kc * 128:(kc + 1) * 128, mo * 128:(mo + 1) * 128])
    for kc in range(2):
        put(f"wy_{kc}", wy[kc * 128:(kc + 1) * 128, :])
    for kc in range(3):
        put(f"wz_{kc}", wz[kc * 128:(kc + 1) * 128, :])
    return img.astype(BF16)


def _pack_biases(bl, gru_bi, gru_bh, conv1_b, conv2_b, convc1_b, convc2_b, by, bz_):
    img = np.zeros((128, bl.col), np.float32)

    def put(name, vec, nch):
        o = bl.off[name]
        for c in range(nch):
            img[:, o + c] = vec[c * 128:(c + 1) * 128]

    put("br", gru_bi[0:256] + gru_bh[0:256], 2)
    put("bz", gru_bi[256:512] + gru_bh[256:512], 2)
    put("big", gru_bi[512:768], 2)
    put("bhg", gru_bh[512:768], 2)
    put("c1b", conv1_b, 2)
    put("c2b", conv2_b, 2)
    put("cc1b", convc1_b, 3)
    put("cc2b", convc2_b, 3)
    img[0, bl.off["by"]] = by[0]
    img[0, bl.off["bz_"]] = bz_[0]
    return img


# ---------------------------------------------------------------------------
# edge preprocessing: sort by dst, bucket into (core, window), chunk by 128
# ---------------------------------------------------------------------------
NB = 4                  # source-node blocks (8192 nodes each) for int16 gather
NSB = NN // NB          # 8192 nodes per block -> local row idx < 32768
GW = 2                  # windows per gather group (keeps each dma_gather at
                        # 1024 descriptors == the SWDGE ring carveout limit)


def _preprocess_edges(src, dst, etype):
    """Group edges by (dst-window, src-block); emit per-core gather index
    streams (int16, dma_gather wrapped layout) and one-hot S images.

    Slot layout: window w's chunks are ordered b-major with uniform per-b
    chunk count CB[b]; gather calls cover (src-block b, group of GW windows).
    """
    core = dst // NPC
    w = (dst % NPC) // WIN
    b = src // NSB
    dloc = (dst % WIN).astype(np.int64)
    lidx = ((src % NSB) * NT + etype).astype(np.int32)   # block-local row
    key = (core * NWIN + w) * NB + b
    order = np.argsort(key, kind="stable")
    cnt = np.bincount(key, minlength=NCORES * NWIN * NB).reshape(NCORES, NWIN, NB)
    CB = np.maximum(1, -(-cnt // 128)).max(axis=(0, 1)).astype(int)  # [NB]
    SCB = int(CB.sum())              # chunks per window
    TOTCH = NWIN * SCB               # chunks per step
    starts = np.zeros(NCORES * NWIN * NB + 1, np.int64)
    starts[1:] = np.cumsum(cnt.reshape(-1))
    # S image: [128, TOTCH*128]; chunk order: (w, b, j)
    S_img = np.zeros((NCORES, 128, TOTCH * 128), np.float32)
    # idx streams: one per (b, group): order (w in group, j, p) chunk-major
    ngrp = NWIN // GW
    idx16 = [np.zeros((NCORES, 128, (NWIN // GW) * (GW * int(CB[bb]) * 128 // 16)),
                      np.int16) for bb in range(NB)]
    ar128 = np.arange(128)
    boff = np.concatenate([[0], np.cumsum(CB)]).astype(int)  # chunk offset of block b in window
    for c in range(NCORES):
        for bb in range(NB):
            nchk_b = int(CB[bb])
            for g in range(ngrp):
                flat = np.zeros(GW * nchk_b * 128, np.int32)
                for wi in range(GW):
                    wdx = g * GW + wi
                    s0 = starts[(c * NWIN + wdx) * NB + bb]
                    s1 = starts[(c * NWIN + wdx) * NB + bb + 1]
                    seg = order[s0:s1]
                    n = len(seg)
                    pad = nchk_b * 128 - n
                    li = np.concatenate([lidx[seg], np.zeros(pad, np.int32)])
                    dl = np.concatenate([dloc[seg], np.full(pad, -1, np.int64)])
                    flat[wi * nchk_b * 128:(wi + 1) * nchk_b * 128] = li
                    # S columns for window wdx, block bb, chunk j
                    dlb = dl.reshape(nchk_b, 128)
                    for j in range(nchk_b):
                        valid = dlb[j] >= 0
                        col = (wdx * SCB + boff[bb] + j) * 128
                        S_img[c][ar128[valid], col + dlb[j][valid]] = 1.0
                # wrap: flat[i] lives at [i % 16, i // 16]
                ncol = (len(flat) + 15) // 16
                wrapped = np.zeros((16, ncol), np.int16)
                wrapped[:, :] = flat.astype(np.int16).reshape(ncol, 16).T
                # replicate over all 128 partitions (p reads p % 16)
                full = np.tile(wrapped, (8, 1))
                cbase = g * ncol  # we store groups side by side per b
                idx16[bb][c][:, cbase:cbase + ncol] = full
    return idx16, S_img.astype(BF16), CB, SCB


# ---------------------------------------------------------------------------
# device program
# ---------------------------------------------------------------------------
def build_program(CB, wl, bl, num_devices=NCORES, sim_mode=False, nsteps=NSTEPS):
    CB = [int(x) for x in CB]
    SCB = int(sum(CB))
    boff = [0]
    for x in CB:
        boff.append(boff[-1] + x)
    TOTCH = NWIN * SCB
    ncol_b = [GW * CB[bb] * 128 // 16 for bb in range(NB)]
    nc = bacc.Bacc("TRN2", target_bir_lowering=False, debug=False,
                   num_devices=num_devices)
    h0T_d = nc.dram_tensor("h0T", [OUT, NPC], bf, kind="ExternalInput")
    xT_d = nc.dram_tensor("xT", [IN_DIM, NPC], bf, kind="ExternalInput")
    Wimg_d = nc.dram_tensor("Wimg", [128, wl.col], bf, kind="ExternalInput")
    Bimg_d = nc.dram_tensor("Bimg", [128, bl.col], f32, kind="ExternalInput")
    idx_d = [nc.dram_tensor(f"idx{bb}", [128, (NWIN // GW) * ncol_b[bb]],
                            mybir.dt.int16, kind="ExternalInput")
             for bb in range(NB)]
    Simg_d = nc.dram_tensor("Simg", [128, TOTCH * 128], bf, kind="ExternalInput")
    out_d = nc.dram_tensor("out", [GPC], f32, kind="ExternalOutput")

    with tile.TileContext(nc) as tc:
        with tc.tile_pool(name="persist", bufs=1) as pp, \
             tc.tile_pool(name="dram", bufs=1, space="DRAM") as dpool:
            hT0 = pp.tile([128, NPC], bf)
            hT1 = pp.tile([128, NPC], bf)
            Wsb = pp.tile([128, wl.col], bf)
            Bsb = pp.tile([128, bl.col], f32)
            idx_sb = [pp.tile([128, (NWIN // GW) * ncol_b[bb]], mybir.dt.int16,
                              name=f"idxsb{bb}") for bb in range(NB)]
            nc.sync.dma_start(hT0[:], h0T_d.ap()[0:128, :])
            nc.sync.dma_start(hT1[:], h0T_d.ap()[128:256, :])
            nc.sync.dma_start(Wsb[:], Wimg_d.ap())
            nc.sync.dma_start(Bsb[:], Bimg_d.ap())
            for bb in range(NB):
                nc.sync.dma_start(idx_sb[bb][:], idx_d[bb].ap())
            hT = [hT0, hT1]

            def W(name):
                o = wl.off[name]
                return Wsb[:, o:o + (OUT if name.startswith("wmsg") else
                                     1 if name.startswith("wy") or name.startswith("wz") else 128)]

            def Bias(name, c=0):
                o = bl.off[name] + c
                return Bsb[:, o:o + 1]

            def Bias1(name):
                o = bl.off[name]
                return Bsb[0:1, o:o + 1]

            # ---------------- GGNN loop ----------------
            with tc.tile_pool(name="stage", bufs=4) as stp, \
                 tc.tile_pool(name="gat", bufs=2) as gap, \
                 tc.tile_pool(name="swin", bufs=3) as swp, \
                 tc.tile_pool(name="gtmp", bufs=2) as gtp, \
                 tc.tile_pool(name="pH", bufs=1, space="PSUM") as pH, \
                 tc.tile_pool(name="pA", bufs=2, space="PSUM") as pA, \
                 tc.tile_pool(name="pG", bufs=1, space="PSUM") as pG:
                for s in range(nsteps):
                    in_cc = dpool.tile([NPC * NT, OUT], bf, name=f"in_cc{s}")
                    out_cc = dpool.tile([NN * NT, OUT], bf, addr_space="Shared",
                                        name=f"out_cc{s}")
                    # --- H_all build: rows (n, t) of h @ W_msg[t], bf16 ---
                    for nch in range(NPC // 128):
                        ps_a = pH.tile([128, 512], f32, space="PSUM", tag="hba")
                        ps_b = pH.tile([128, 512], f32, space="PSUM", tag="hbb")
                        for t in range(NT):
                            ps = ps_a if t < 2 else ps_b
                            o = (t % 2) * 256
                            for kc in range(2):
                                nc.tensor.matmul(
                                    ps[:, o:o + 256],
                                    lhsT=hT[kc][:, nch * 128:(nch + 1) * 128],
                                    rhs=W(f"wmsg_{t}_{kc}"),
                                    start=(kc == 0), stop=(kc == 1))
                        stg = stp.tile([128, NT * OUT], bf, tag="stg")
                        for t in range(NT):
                            psrc = (ps_a if t < 2 else ps_b)[:, (t % 2) * 256:(t % 2) * 256 + 256]
                            if t % 2 == 0:
                                nc.vector.tensor_copy(stg[:, t * 256:(t + 1) * 256], psrc)
                            else:
                                nc.scalar.activation(stg[:, t * 256:(t + 1) * 256],
                                                     psrc, AF.Copy)
                        nc.sync.dma_start(
                            in_cc[nch * 512:(nch + 1) * 512, :], stg[:])
                    # --- AllGather the message table ---
                    if sim_mode:
                        nc.sync.dma_start(out_cc[0:NPC * NT, :], in_cc[:])
                    else:
                        nc.gpsimd.collective_compute(
                            "AllGather", ALU.bypass,
                            replica_groups=[list(range(num_devices))],
                            ins=[in_cc.opt()],
                            outs=[out_cc.opt()],
                        )
                    # --- per window-group: dma_gather per src-block, scatter,
                    #     GRU on 256-node blocks (group == GRU block, GW=2) ---
                    for grp in range(NWIN // GW):
                      gts = []
                      for bb in range(NB):
                          gt_b = gap.tile([128, GW * CB[bb], OUT], bf,
                                          tag=f"g{bb}", name=f"g{bb}")
                          nc.gpsimd.dma_gather(
                              gt_b[:, :, :],
                              out_cc[bb * NSB * NT:(bb + 1) * NSB * NT, :],
                              idx_sb[bb][:, grp * ncol_b[bb]:(grp + 1) * ncol_b[bb]],
                              GW * CB[bb] * 128,
                              GW * CB[bb] * 128,
                              OUT)
                          gts.append(gt_b)
                      Swin = swp.tile([128, GW * SCB * 128], bf, tag="swin")
                      nc.sync.dma_start(
                          Swin[:],
                          Simg_d.ap()[:, grp * GW * SCB * 128:(grp + 1) * GW * SCB * 128])
                      for sub in range(GW // 2):
                        aTs = [gtp.tile([128, 256], bf, tag="aT0", name="aT0"),
                               gtp.tile([128, 256], bf, tag="aT1", name="aT1")]
                        for wi2 in range(2):
                            wi = sub * 2 + wi2
                            ps_aT = pA.tile([128, 256], f32, space="PSUM", tag="aT")
                            for fc in range(2):
                                nmm = 0
                                for bb in range(NB):
                                    for j in range(CB[bb]):
                                        scol = (wi * SCB + boff[bb] + j) * 128
                                        nc.tensor.matmul(
                                            ps_aT[:, fc * 128:(fc + 1) * 128],
                                            lhsT=gts[bb][:, wi * CB[bb] + j,
                                                         fc * 128:(fc + 1) * 128],
                                            rhs=Swin[:, scol:scol + 128],
                                            start=(nmm == 0), stop=(nmm == SCB - 1))
                                        nmm += 1
                            nc.vector.tensor_copy(aTs[0][:, wi2 * 128:(wi2 + 1) * 128],
                                                  ps_aT[:, 0:128])
                            nc.vector.tensor_copy(aTs[1][:, wi2 * 128:(wi2 + 1) * 128],
                                                  ps_aT[:, 128:256])
                        # GRU on nodes [(grp*GW + sub*2)*128, +256)
                        nb = (grp * GW + sub * 2) * 128
                        hw = [hT[0][:, nb:nb + 256], hT[1][:, nb:nb + 256]]
                        psg = []
                        for fc in range(2):
                            ps = pG.tile([128, 1024], f32, space="PSUM", tag=f"g{fc}")
                            psg.append(ps)
                            # cols: r 0:256 | z 256:512 | ig 512:768 | hg 768:1024
                            for gi in (0, 1):
                                col = gi * 256
                                mi = gi * 2 + fc
                                nc.tensor.matmul(ps[:, col:col + 256],
                                                 lhsT=W(f"wi_{mi}_0"), rhs=aTs[0][:],
                                                 start=True, stop=False)
                                nc.tensor.matmul(ps[:, col:col + 256],
                                                 lhsT=W(f"wi_{mi}_1"), rhs=aTs[1][:],
                                                 start=False, stop=False)
                                nc.tensor.matmul(ps[:, col:col + 256],
                                                 lhsT=W(f"wh_{mi}_0"), rhs=hw[0],
                                                 start=False, stop=False)
                                nc.tensor.matmul(ps[:, col:col + 256],
                                                 lhsT=W(f"wh_{mi}_1"), rhs=hw[1],
                                                 start=False, stop=True)
                            mi = 4 + fc
                            nc.tensor.matmul(ps[:, 512:768], lhsT=W(f"wi_{mi}_0"),
                                             rhs=aTs[0][:], start=True, stop=False)
                            nc.tensor.matmul(ps[:, 512:768], lhsT=W(f"wi_{mi}_1"),
                                             rhs=aTs[1][:], start=False, stop=True)
                            nc.tensor.matmul(ps[:, 768:1024], lhsT=W(f"wh_{mi}_0"),
                                             rhs=hw[0], start=True, stop=False)
                            nc.tensor.matmul(ps[:, 768:1024], lhsT=W(f"wh_{mi}_1"),
                                             rhs=hw[1], start=False, stop=True)
                        # gates + state update ([128, 256] tiles)
                        for fc in range(2):
                            ps = psg[fc]
                            hslice = hT[fc][:, nb:nb + 256]
                            r = gtp.tile([128, 256], bf, tag="r")
                            z = gtp.tile([128, 256], bf, tag="z")
                            t1 = gtp.tile([128, 256], f32, tag="t1")
                            g = gtp.tile([128, 256], f32, tag="g2")
                            d = gtp.tile([128, 256], f32, tag="d")
                            nc.scalar.activation(r[:], ps[:, 0:256], AF.Sigmoid,
                                                 bias=Bias("br", fc))
                            nc.scalar.activation(z[:], ps[:, 256:512], AF.Sigmoid,
                                                 bias=Bias("bz", fc))
                            # t1 = (hg + bhg) * r
                            nc.vector.scalar_tensor_tensor(
                                t1[:], ps[:, 768:1024], Bias("bhg", fc), r[:],
                                op0=ALU.add, op1=ALU.mult)
                            nc.vector.tensor_add(t1[:], t1[:], ps[:, 512:768])
                            nc.scalar.activation(g[:], t1[:], AF.Tanh,
                                                 bias=Bias("big", fc))
                            nc.vector.tensor_sub(d[:], hslice, g[:])
                            nc.vector.tensor_mul(d[:], z[:], d[:])
                            nc.vector.tensor_add(hslice, g[:], d[:])

            # ---------------- readout ----------------
            with tc.tile_pool(name="rsb", bufs=2) as rsb, \
                 tc.tile_pool(name="rx", bufs=1) as rxp, \
                 tc.tile_pool(name="pR", bufs=1, space="PSUM") as pR, \
                 tc.tile_pool(name="pV", bufs=1, space="PSUM") as pV:
                res_sb = pp.tile([1, GPC], f32)
                xTb = rxp.tile([128, NPC], bf)
                nc.sync.dma_start(xTb[:], xT_d.ap())

                def pool_step(y, width, ksz, tag):
                    # maxpool stride 2 over free dim, VALID
                    outw = (width - ksz) // 2 + 1
                    pout = rsb.tile([128, outw], bf, tag=tag)
                    ab = y[:, 0:2 * outw].rearrange("p (n t) -> p n t", t=2)
                    a0 = ab[:, :, 0]
                    a1 = ab[:, :, 1]
                    if ksz == 3:
                        tmp = rsb.tile([128, outw], bf, tag=tag + "_t")
                        nc.vector.tensor_max(tmp[:], a0, a1)
                        a2 = y[:, 2:2 * outw + 2].rearrange("p (n t) -> p n t", t=2)[:, :, 0]
                        nc.vector.tensor_max(pout[:], tmp[:], a2)
                    else:
                        nc.vector.tensor_max(pout[:], a0, a1)
                    return pout

                for gidx in range(GPC):
                    g0 = gidx * LG
                    # ---- Y path (h only, 256 ch) ----
                    y2p = []
                    for mo in range(2):
                        psY = pR.tile([128, 254], f32, space="PSUM", tag="psY")
                        n_mm = 0
                        for tap in range(3):
                            for kc in range(2):
                                nc.tensor.matmul(
                                    psY[:],
                                    lhsT=W(f"c1_{tap}_{kc}_{mo}"),
                                    rhs=hT[kc][:, g0 + tap:g0 + tap + 254],
                                    start=(n_mm == 0), stop=(n_mm == 5))
                                n_mm += 1
                        y1 = rsb.tile([128, 254], bf, tag=f"y1_{mo}")
                        nc.scalar.activation(y1[:], psY[:], AF.Relu,
                                             bias=Bias("c1b", mo))
                        y2p.append(pool_step(y1, 254, 3, f"p3_{mo}"))
                    y3p = []
                    for mo in range(2):
                        psY2 = pR.tile([128, 126], f32, space="PSUM", tag="psY2")
                        for kc in range(2):
                            nc.tensor.matmul(psY2[:], lhsT=W(f"c2_{kc}_{mo}"),
                                             rhs=y2p[kc][:],
                                             start=(kc == 0), stop=(kc == 1))
                        y2 = rsb.tile([128, 126], bf, tag=f"y2_{mo}")
                        nc.scalar.activation(y2[:], psY2[:], AF.Relu,
                                             bias=Bias("c2b", mo))
                        y3p.append(pool_step(y2, 126, 2, f"yp_{mo}"))
                    psy = pV.tile([1, 63], f32, space="PSUM", tag="psy")
                    for kc in range(2):
                        nc.tensor.matmul(psy[:], lhsT=W(f"wy_{kc}"),
                                         rhs=y3p[kc][:],
                                         start=(kc == 0), stop=(kc == 1))
                    # ---- Z path (concat h|x, 384 ch) ----
                    cch = [hT[0], hT[1], xTb]
                    z2p = []
                    for mo in range(3):
                        psZ = pR.tile([128, 254], f32, space="PSUM", tag="psZ")
                        n_mm = 0
                        for tap in range(3):
                            for kc in range(3):
                                nc.tensor.matmul(
                                    psZ[:],
                                    lhsT=W(f"cc1_{tap}_{kc}_{mo}"),
                                    rhs=cch[kc][:, g0 + tap:g0 + tap + 254],
                                    start=(n_mm == 0), stop=(n_mm == 8))
                                n_mm += 1
                        z1 = rsb.tile([128, 254], bf, tag=f"z1_{mo}")
                        nc.scalar.activation(z1[:], psZ[:], AF.Relu,
                                             bias=Bias("cc1b", mo))
                        z2p.append(pool_step(z1, 254, 3, f"zp_{mo}"))
                    z3p = []
                    for mo in range(3):
                        psZ2 = pR.tile([128, 126], f32, space="PSUM", tag="psZ2")
                        for kc in range(3):
                            nc.tensor.matmul(psZ2[:], lhsT=W(f"cc2_{kc}_{mo}"),
                                             rhs=z2p[kc][:],
                                             start=(kc == 0), stop=(kc == 2))
                        z2 = rsb.tile([128, 126], bf, tag=f"z2_{mo}")
                        nc.scalar.activation(z2[:], psZ2[:], AF.Relu,
                                             bias=Bias("cc2b", mo))
                        z3p.append(pool_step(z2, 126, 2, f"zq_{mo}"))
                    psz = pV.tile([1, 63], f32, space="PSUM", tag="psz")
                    for kc in range(3):
                        nc.tensor.matmul(psz[:], lhsT=W(f"wz_{kc}"),
                                         rhs=z3p[kc][:],
                                         start=(kc == 0), stop=(kc == 2))
                    # ---- combine ----
                    ty = rsb.tile([1, 63], f32, tag="ty")
                    tz = rsb.tile([1, 63], f32, tag="tz")
                    pr = rsb.tile([1, 63], f32, tag="pr")
                    sm = rsb.tile([1, 1], f32, tag="sm")
                    nc.vector.tensor_scalar_add(ty[:], psy[:], Bias1("by"))
                    nc.vector.tensor_scalar_add(tz[:], psz[:], Bias1("bz_"))
                    nc.vector.tensor_mul(pr[:], ty[:], tz[:])
                    nc.vector.tensor_reduce(sm[:], pr[:],
                                            axis=mybir.AxisListType.X, op=ALU.add)
                    nc.scalar.activation(res_sb[0:1, gidx:gidx + 1], sm[:],
                                         AF.Sigmoid, scale=1.0 / 63.0)
                nc.sync.dma_start(out_d.ap(), res_sb[0:1, :])
    nc.finalize()
    return nc


# ---------------------------------------------------------------------------
# host entry
# ---------------------------------------------------------------------------
def _prepare(inputs):
    features = np.asarray(inputs["features"], np.float32)
    src = np.asarray(inputs["src"]).astype(np.int64)
    dst = np.asarray(inputs["dst"]).astype(np.int64)
    etype = np.asarray(inputs["etype"]).astype(np.int64)
    wl = _make_wlayout()
    bl = _make_blayout()
    Wimg = _pack_weights(
        wl,
        np.asarray(inputs["W_msg"], np.float32),
        np.asarray(inputs["gru_Wi"], np.float32),
        np.asarray(inputs["gru_Wh"], np.float32),
        np.asarray(inputs["conv1_w"], np.float32),
        np.asarray(inputs["conv2_w"], np.float32),
        np.asarray(inputs["convc1_w"], np.float32),
        np.asarray(inputs["convc2_w"], np.float32),
        np.asarray(inputs["wy"], np.float32),
        np.asarray(inputs["wz"], np.float32),
    )
    Bimg = _pack_biases(
        bl,
        np.asarray(inputs["gru_bi"], np.float32),
        np.asarray(inputs["gru_bh"], np.float32),
        np.asarray(inputs["conv1_b"], np.float32),
        np.asarray(inputs["conv2_b"], np.float32),
        np.asarray(inputs["convc1_b"], np.float32),
        np.asarray(inputs["convc2_b"], np.float32),
        np.asarray(inputs["by"], np.float32),
        np.asarray(inputs["bz"], np.float32),
    )
    # note: b_msg is folded into nothing here -- reference setup has zeros.
    # (general-case support: a nonzero b_msg would need a degree-weighted
    # bias added to aT; assert instead so failures are loud.)
    b_msg = np.asarray(inputs["b_msg"], np.float32)
    assert np.abs(b_msg).max() == 0.0, "nonzero b_msg not supported"

    idx16, S_img, CB, SCB = _preprocess_edges(src, dst, etype)

    in_maps = []
    for c in range(NCORES):
        feats = features[c * NPC:(c + 1) * NPC]  # [4096, 128]
        xT = feats.T.astype(BF16)                # [128, 4096]
        h0T = np.zeros((OUT, NPC), np.float32)
        h0T[:IN_DIM] = feats.T
        im = {
            "h0T": h0T.astype(BF16),
            "xT": xT,
            "Wimg": Wimg,
            "Bimg": Bimg,
            "Simg": S_img[c],
        }
        for bb in range(NB):
            im[f"idx{bb}"] = idx16[bb][c]
        in_maps.append(im)
    return wl, bl, CB, in_maps


def kernel(**inputs):
    wl, bl, CB, in_maps = _prepare(inputs)
    nc = build_program(CB, wl, bl)
    res = run_bass_kernel_spmd(nc, in_maps, core_ids=list(range(NCORES)))
    out = np.concatenate([res.results[c]["out"] for c in range(NCORES)])
    return out.astype(np.float32)



# revision 3
# speedup vs baseline: 2.3629x; 2.3629x over previous
"""Devign GGNN model on 8 Trainium2 NeuronCores (Bass/Tile) — v4.

Aggregate-first GGNN step (vs the prior transform-first kernel):
  a[v] = sum_t ( sum_{e: dst=v, et=t} h[src_e] ) @ W_msg[t]
so the collective moves raw h ([32768,256] bf16, 16.8MB/step) instead of
the 4x-larger per-etype transformed table (67MB/step).

Per step (pipelined per dst-window of 128 nodes):
  1. dma_gather h[src] rows from the allgathered table (edges bucketed by
     (window, etype, src-half), 128-slot chunks, <=1024 desc per call)
  2. one-hot scatter matmul into per-etype A_t psum [feat, dst]
  3. W_msg apply (PE) -> aT; GRU for the window (PE + ACT/DVE), lagging
     the scatter by one window so PSUM->SBUF copies hide under PE work
  4. transpose updated hT chunk -> node-major rows of the next step's
     table (PE transpose); AllGather fires per src-half as soon as that
     half of the table is built, overlapping the next step's collective
     with the current step's compute
  5. readout (conv/pool stacks as tap-shifted matmuls) unchanged.
"""
import sys
import numpy as np

for _p in ("/opt/trn_rl_repo",):
    if _p not in sys.path:
        sys.path.insert(0, _p)

import ml_dtypes

import concourse.bass as bass
import concourse.mybir as mybir
import concourse.tile as tile
from concourse import bacc
from concourse.bass_utils import run_bass_kernel_spmd

BF16 = ml_dtypes.bfloat16
F32 = np.float32

NCORES = 8
NN = 32768          # total nodes
IN_DIM = 128
OUT = 256
NT = 4              # edge types
NSTEPS = 8
NGRAPH = 128
NPC = NN // NCORES  # nodes per core = 4096
WIN = 128           # dst window size
NWIN = NPC // WIN   # 32 windows per core
GPC = NGRAPH // NCORES  # graphs per core = 16
LG = 256            # nodes per graph
CONCAT = IN_DIM + OUT  # 384
NHALF = 2           # src-halves per AllGather (collective chunking)

bf = mybir.dt.bfloat16
f32 = mybir.dt.float32
i32 = mybir.dt.int32
AF = mybir.ActivationFunctionType
ALU = mybir.AluOpType


# ---------------------------------------------------------------------------
# weight/bias image layout (shared between host packer and device slicer)
# ---------------------------------------------------------------------------
class WLayout:
    def __init__(self):
        self.col = 0
        self.off = {}

    def alloc(self, name, width):
        self.off[name] = self.col
        self.col += width
        return self.off[name]


def _make_wlayout():
    wl = WLayout()
    for t in range(NT):
        for fi in range(2):
            for fo in range(2):
                wl.alloc(f"wmsgT_{t}_{fi}_{fo}", 128)  # lhsT blocks [fi, fo]
    for g in range(6):
        for kc in range(2):
            wl.alloc(f"wi_{g}_{kc}", 128)          # lhsT blocks
            wl.alloc(f"wh_{g}_{kc}", 128)
    for tap in range(3):
        for kc in range(2):
            for mo in range(2):
                wl.alloc(f"c1_{tap}_{kc}_{mo}", 128)
    for kc in range(2):
        for mo in range(2):
            wl.alloc(f"c2_{kc}_{mo}", 128)
    for tap in range(3):
        for kc in range(3):
            for mo in range(3):
                wl.alloc(f"cc1_{tap}_{kc}_{mo}", 128)
    for kc in range(3):
        for mo in range(3):
            wl.alloc(f"cc2_{kc}_{mo}", 128)
    for kc in range(2):
        wl.alloc(f"wy_{kc}", 1)
    for kc in range(3):
        wl.alloc(f"wz_{kc}", 1)
    wl.alloc("ident", 128)
    return wl


def _make_blayout():
    bl = WLayout()
    for name in ("br", "bz", "big", "bhg"):
        bl.alloc(name, 2)       # [256] as two [128] cols
    bl.alloc("c1b", 2)
    bl.alloc("c2b", 2)
    bl.alloc("cc1b", 3)
    bl.alloc("cc2b", 3)
    bl.alloc("by", 1)
    bl.alloc("bz_", 1)
    return bl


def _pack_weights(wl, W_msg, gru_Wi, gru_Wh, conv1_w, conv2_w, convc1_w, convc2_w, wy, wz):
    img = np.zeros((128, wl.col), np.float32)

    def put(name, block):
        o = wl.off[name]
        img[:, o:o + block.shape[1]] = block

    for t in range(NT):
        for fi in range(2):
            for fo in range(2):
                put(f"wmsgT_{t}_{fi}_{fo}",
                    W_msg[t][fi * 128:(fi + 1) * 128, fo * 128:(fo + 1) * 128])
    for g in range(6):
        for kc in range(2):
            put(f"wi_{g}_{kc}", gru_Wi[kc * 128:(kc + 1) * 128, g * 128:(g + 1) * 128])
            put(f"wh_{g}_{kc}", gru_Wh[kc * 128:(kc + 1) * 128, g * 128:(g + 1) * 128])
    for tap in range(3):
        w_t = conv1_w[:, :, tap].T  # [i, o]
        for kc in range(2):
            for mo in range(2):
                put(f"c1_{tap}_{kc}_{mo}", w_t[kc * 128:(kc + 1) * 128, mo * 128:(mo + 1) * 128])
    w2 = conv2_w[:, :, 0].T
    for kc in range(2):
        for mo in range(2):
            put(f"c2_{kc}_{mo}", w2[kc * 128:(kc + 1) * 128, mo * 128:(mo + 1) * 128])
    for tap in range(3):
        w_t = convc1_w[:, :, tap].T
        for kc in range(3):
            for mo in range(3):
                put(f"cc1_{tap}_{kc}_{mo}", w_t[kc * 128:(kc + 1) * 128, mo * 128:(mo + 1) * 128])
    wc2 = convc2_w[:, :, 0].T
    for kc in range(3):
        for mo in range(3):
            put(f"cc2_{kc}_{mo}", wc2[kc * 128:(kc + 1) * 128, mo * 128:(mo + 1) * 128])
    for kc in range(2):
        put(f"wy_{kc}", wy[kc * 128:(kc + 1) * 128, :])
    for kc in range(3):
        put(f"wz_{kc}", wz[kc * 128:(kc + 1) * 128, :])
    put("ident", np.eye(128, dtype=np.float32))
    return img.astype(BF16)


def _pack_biases(bl, gru_bi, gru_bh, conv1_b, conv2_b, convc1_b, convc2_b, by, bz_):
    img = np.zeros((128, bl.col), np.float32)

    def put(name, vec, nch):
        o = bl.off[name]
        for c in range(nch):
            img[:, o + c] = vec[c * 128:(c + 1) * 128]

    put("br", gru_bi[0:256] + gru_bh[0:256], 2)
    put("bz", gru_bi[256:512] + gru_bh[256:512], 2)
    put("big", gru_bi[512:768], 2)
    put("bhg", gru_bh[512:768], 2)
    put("c1b", conv1_b, 2)
    put("c2b", conv2_b, 2)
    put("cc1b", convc1_b, 3)
    put("cc2b", convc2_b, 3)
    img[0, bl.off["by"]] = by[0]
    img[0, bl.off["bz_"]] = bz_[0]
    return img


# ---------------------------------------------------------------------------
# edge preprocessing: bucket by (core, dst-window, etype[, src-half]);
# chunks of 128 edge slots, uniform CB chunks per bucket across all cores.
# ---------------------------------------------------------------------------
def _preprocess_edges(src, dst, etype, nhalf=NHALF):
    core = dst // NPC
    w = (dst % NPC) // WIN
    dloc = (dst % WIN).astype(np.int64)
    hlf = (src % NPC) // (NPC // nhalf)
    # gather-table row index for a src under half-chunked AllGather:
    # out_cc_h rows = [core0 rows of half h | core1 rows ... ]
    rows_per_core = NPC // nhalf
    lidx = ((src // NPC) * rows_per_core + (src % NPC) % rows_per_core).astype(np.int32)
    key = ((core * NWIN + w) * NT + etype) * nhalf + hlf
    nbuck = NCORES * NWIN * NT * nhalf
    order = np.argsort(key, kind="stable")
    cnt = np.bincount(key, minlength=nbuck)
    CB = int(-(-cnt.max() // 128))          # chunks per bucket, uniform
    SCB = NT * nhalf * CB                   # chunks per window
    starts = np.zeros(nbuck + 1, np.int64)
    starts[1:] = np.cumsum(cnt)
    # S image: [128, NWIN*SCB*128] per core; chunk (w, t, h, j) at column
    # ((w*SCB + (t*nhalf+h)*CB + j) * 128
    S_img = np.zeros((NCORES, 128, NWIN * SCB * 128), np.float32)
    # idx stream: per (c, w, h): NT*CB*128 int16 indices, dma_gather wrap
    call_len = NT * CB * 128
    ncol_call = call_len // 16
    idx16 = np.zeros((NCORES, 128, NWIN * nhalf * ncol_call), np.int16)
    ar128 = np.arange(128)
    for c in range(NCORES):
        for wdx in range(NWIN):
            for h in range(nhalf):
                flat = np.zeros(call_len, np.int32)
                for t in range(NT):
                    b = ((c * NWIN + wdx) * NT + t) * nhalf + h
                    seg = order[starts[b]:starts[b + 1]]
                    n = len(seg)
                    pad = CB * 128 - n
                    li = np.concatenate([lidx[seg], np.zeros(pad, np.int32)])
                    dl = np.concatenate([dloc[seg], np.full(pad, -1, np.int64)])
                    flat[t * CB * 128:(t + 1) * CB * 128] = li
                    dlb = dl.reshape(CB, 128)
                    for j in range(CB):
                        valid = dlb[j] >= 0
                        col = (wdx * SCB + (t * nhalf + h) * CB + j) * 128
                        S_img[c][ar128[valid], col + dlb[j][valid]] = 1.0
                wrapped = flat.astype(np.int16).reshape(ncol_call, 16).T
                full = np.tile(wrapped, (8, 1))
                cbase = (wdx * nhalf + h) * ncol_call
                idx16[c][:, cbase:cbase + ncol_call] = full
    return idx16, S_img.astype(BF16), CB


# ---------------------------------------------------------------------------
# device program
# ---------------------------------------------------------------------------
def build_program(CB, wl, bl, num_devices=NCORES, sim_mode=False, nsteps=NSTEPS,
                  nhalf=NHALF, skip_gather=False, skip_scatter=False,
                  skip_gru=False, skip_swin=False, skip_table=False):
    CB = int(CB)
    SCB = NT * nhalf * CB
    call_len = NT * CB * 128
    ncol_call = call_len // 16
    rows_per_core = NPC // nhalf
    nc = bacc.Bacc("TRN2", target_bir_lowering=False, debug=False,
                   num_devices=num_devices)
    h0T_d = nc.dram_tensor("h0T", [OUT, NPC], bf, kind="ExternalInput")
    xT_d = nc.dram_tensor("xT", [IN_DIM, NPC], bf, kind="ExternalInput")
    Wimg_d = nc.dram_tensor("Wimg", [128, wl.col], bf, kind="ExternalInput")
    Bimg_d = nc.dram_tensor("Bimg", [128, bl.col], f32, kind="ExternalInput")
    idx_d = nc.dram_tensor("idx", [128, NWIN * nhalf * ncol_call],
                           mybir.dt.int16, kind="ExternalInput")
    Simg_d = nc.dram_tensor("Simg", [128, NWIN * SCB * 128], bf, kind="ExternalInput")
    out_d = nc.dram_tensor("out", [GPC], f32, kind="ExternalOutput")

    with tile.TileContext(nc) as tc:
        with tc.tile_pool(name="persist", bufs=1) as pp, \
             tc.tile_pool(name="dram", bufs=1, space="DRAM") as dpool:
            hT0 = pp.tile([128, NPC], bf)
            hT1 = pp.tile([128, NPC], bf)
            Wsb = pp.tile([128, wl.col], bf)
            Bsb = pp.tile([128, bl.col], f32)
            idx_sb = pp.tile([128, NWIN * nhalf * ncol_call], mybir.dt.int16)
            nc.sync.dma_start(hT0[:], h0T_d.ap()[0:128, :])
            nc.sync.dma_start(hT1[:], h0T_d.ap()[128:256, :])
            nc.sync.dma_start(Wsb[:], Wimg_d.ap())
            nc.sync.dma_start(Bsb[:], Bimg_d.ap())
            nc.sync.dma_start(idx_sb[:], idx_d.ap())
            hT = [hT0, hT1]

            def W(name):
                o = wl.off[name]
                w_ = 1 if name.startswith("wy") or name.startswith("wz") else 128
                return Wsb[:, o:o + w_]

            def Bias(name, c=0):
                o = bl.off[name] + c
                return Bsb[:, o:o + 1]

            def Bias1(name):
                o = bl.off[name]
                return Bsb[0:1, o:o + 1]

            # ---------------- GGNN loop ----------------
            with tc.tile_pool(name="stage", bufs=6) as stp, \
                 tc.tile_pool(name="gat", bufs=4) as gap, \
                 tc.tile_pool(name="swin", bufs=5) as swp, \
                 tc.tile_pool(name="gtmp", bufs=3) as gtp, \
                 tc.tile_pool(name="pA", bufs=1, space="PSUM") as pA, \
                 tc.tile_pool(name="pG", bufs=1, space="PSUM") as pG:
                def build_table_chunk(in_cc, nch):
                    stg = stp.tile([128, OUT], bf, tag="stg")
                    ps_t = pG.tile([128, 256], bf, space="PSUM", tag="pt")
                    for fc in range(2):
                        nc.tensor.transpose(
                            ps_t[:, fc * 128:(fc + 1) * 128],
                            hT[fc][:, nch * 128:(nch + 1) * 128],
                            W("ident"))
                    nc.vector.tensor_copy(stg[:], ps_t[:])
                    nc.sync.dma_start(in_cc[nch * 128:(nch + 1) * 128, :], stg[:])

                def fire_ag(in_cc, out_cc, h):
                    seg = in_cc[h * rows_per_core:(h + 1) * rows_per_core, :]
                    if sim_mode:
                        nc.sync.dma_start(out_cc[h][0:rows_per_core, :], seg)
                    else:
                        nc.gpsimd.collective_compute(
                            "AllGather", ALU.bypass,
                            replica_groups=[list(range(num_devices))],
                            ins=[seg.opt()],
                            outs=[out_cc[h].opt()],
                        )

                # prologue: initial table from h0 + both AllGather chunks
                in_cc = dpool.tile([NPC, OUT], bf, name="in_cc0")
                out_cc = [dpool.tile([NN // nhalf, OUT], bf, addr_space="Shared",
                                     name=f"out_cc0_{h}") for h in range(nhalf)]
                if not skip_table:
                    for nch in range(NPC // 128):
                        build_table_chunk(in_cc, nch)
                for h in range(nhalf):
                    fire_ag(in_cc, out_cc, h)

                blocks_per_half = (NWIN // 2) // nhalf
                for s in range(nsteps):
                    last = s == nsteps - 1
                    if not last:
                        in_nxt = dpool.tile([NPC, OUT], bf, name=f"in_cc{s + 1}")
                        out_nxt = [dpool.tile([NN // nhalf, OUT], bf,
                                              addr_space="Shared",
                                              name=f"out_cc{s + 1}_{h}")
                                   for h in range(nhalf)]
                    # --- pipelined per-window loop: GRU lags scatter by
                    #     one window so PSUM->SBUF copies hide under PE work ---
                    nseg = max(1, call_len // 1024)
                    seg_len = call_len // nseg
                    seg_chunks = seg_len // 128
                    seg_ncol = seg_len // 16
                    wins_per_half = NWIN // nhalf
                    pend = None   # (wdx, aTs) awaiting GRU

                    def emit_window_front(wdx):
                        # gathers + Swin + scatter + Atsb copies
                        gts = []
                        for h in range(nhalf):
                            for sg in range(nseg):
                                gt = gap.tile([128, seg_chunks, OUT], bf,
                                              tag=f"g{h}_{sg}", name=f"g{h}_{sg}")
                                o = (wdx * nhalf + h) * ncol_call + sg * seg_ncol
                                nc.gpsimd.dma_gather(
                                    gt[:, :, :],
                                    out_cc[h][:, :],
                                    idx_sb[:, o:o + seg_ncol],
                                    seg_len, seg_len, OUT)
                                gts.append(gt)
                        Swin = swp.tile([128, SCB * 128], bf, tag="swin")
                        nc.sync.dma_start(
                            Swin[:],
                            Simg_d.ap()[:, wdx * SCB * 128:(wdx + 1) * SCB * 128])
                        psA = pA.tile([128, NT * 256 + 256], f32, space="PSUM",
                                      tag="psA")
                        for t in range(NT):
                            for fc in range(2):
                                nmm = 0
                                for h in range(nhalf):
                                    for j in range(CB):
                                        scol = ((t * nhalf + h) * CB + j) * 128
                                        ci = t * CB + j
                                        gtile = gts[h * nseg + ci // seg_chunks]
                                        nc.tensor.matmul(
                                            psA[:, (t * 2 + fc) * 128:(t * 2 + fc + 1) * 128],
                                            lhsT=gtile[:, ci % seg_chunks,
                                                       fc * 128:(fc + 1) * 128],
                                            rhs=Swin[:, scol:scol + 128],
                                            start=(nmm == 0),
                                            stop=(nmm == nhalf * CB - 1))
                                        nmm += 1
                        Atsb = gtp.tile([128, NT * 256], bf, tag="At")
                        for t in range(NT):
                            for fc in range(2):
                                sl = slice((t * 2 + fc) * 128, (t * 2 + fc + 1) * 128)
                                if (t * 2 + fc) % 2 == 0:
                                    nc.vector.tensor_copy(Atsb[:, sl], psA[:, sl])
                                else:
                                    nc.scalar.activation(Atsb[:, sl], psA[:, sl],
                                                         AF.Copy)
                        return psA, Atsb

                    def emit_window_back(wdx, psA, Atsb):
                        # W-apply + aT copies
                        psa = psA[:, 1024:1280]
                        for fo in range(2):
                            nmm = 0
                            for t in range(NT):
                                for fi in range(2):
                                    nc.tensor.matmul(
                                        psa[:, fo * 128:(fo + 1) * 128],
                                        lhsT=W(f"wmsgT_{t}_{fi}_{fo}"),
                                        rhs=Atsb[:, (t * 2 + fi) * 128:
                                                 (t * 2 + fi + 1) * 128],
                                        start=(nmm == 0), stop=(nmm == 7))
                                    nmm += 1
                        aTs = [gtp.tile([128, 128], bf, tag="aT0", name="aT0"),
                               gtp.tile([128, 128], bf, tag="aT1", name="aT1")]
                        nc.vector.tensor_copy(aTs[0][:], psa[:, 0:128])
                        nc.scalar.activation(aTs[1][:], psa[:, 128:256], AF.Copy)
                        return aTs

                    def emit_gru(wdx, aTs):
                        # GRU on the window's 128 nodes; psum [128,1024]:
                        # fc0: r|z|ig|hg at 128 cols each, fc1: +512
                        nb = wdx * 128
                        hw = [hT[0][:, nb:nb + 128], hT[1][:, nb:nb + 128]]
                        ps = pG.tile([128, 1024], f32, space="PSUM", tag="g", bufs=2)
                        for fc in range(2):
                            base = fc * 512
                            for gi in (0, 1):
                                col = base + gi * 128
                                mi = gi * 2 + fc
                                nc.tensor.matmul(ps[:, col:col + 128],
                                                 lhsT=W(f"wi_{mi}_0"), rhs=aTs[0][:],
                                                 start=True, stop=False)
                                nc.tensor.matmul(ps[:, col:col + 128],
                                                 lhsT=W(f"wi_{mi}_1"), rhs=aTs[1][:],
                                                 start=False, stop=False)
                                nc.tensor.matmul(ps[:, col:col + 128],
                                                 lhsT=W(f"wh_{mi}_0"), rhs=hw[0],
                                                 start=False, stop=False)
                                nc.tensor.matmul(ps[:, col:col + 128],
                                                 lhsT=W(f"wh_{mi}_1"), rhs=hw[1],
                                                 start=False, stop=True)
                            mi = 4 + fc
                            nc.tensor.matmul(ps[:, base + 256:base + 384],
                                             lhsT=W(f"wi_{mi}_0"),
                                             rhs=aTs[0][:], start=True, stop=False)
                            nc.tensor.matmul(ps[:, base + 256:base + 384],
                                             lhsT=W(f"wi_{mi}_1"),
                                             rhs=aTs[1][:], start=False, stop=True)
                            nc.tensor.matmul(ps[:, base + 384:base + 512],
                                             lhsT=W(f"wh_{mi}_0"),
                                             rhs=hw[0], start=True, stop=False)
                            nc.tensor.matmul(ps[:, base + 384:base + 512],
                                             lhsT=W(f"wh_{mi}_1"),
                                             rhs=hw[1], start=False, stop=True)
                        for fc in range(2):
                            base = fc * 512
                            hslice = hT[fc][:, nb:nb + 128]
                            r = gtp.tile([128, 128], bf, tag="r")
                            z = gtp.tile([128, 128], bf, tag="z")
                            t1 = gtp.tile([128, 128], f32, tag="t1")
                            g = gtp.tile([128, 128], f32, tag="g2")
                            d = gtp.tile([128, 128], f32, tag="d")
                            nc.scalar.activation(r[:], ps[:, base:base + 128],
                                                 AF.Sigmoid, bias=Bias("br", fc))
                            nc.scalar.activation(z[:], ps[:, base + 128:base + 256],
                                                 AF.Sigmoid, bias=Bias("bz", fc))
                            nc.vector.scalar_tensor_tensor(
                                t1[:], ps[:, base + 384:base + 512], Bias("bhg", fc),
                                r[:], op0=ALU.add, op1=ALU.mult)
                            nc.vector.tensor_add(t1[:], t1[:],
                                                 ps[:, base + 256:base + 384])
                            nc.scalar.activation(g[:], t1[:], AF.Tanh,
                                                 bias=Bias("big", fc))
                            nc.vector.tensor_sub(d[:], hslice, g[:])
                            nc.vector.tensor_mul(d[:], z[:], d[:])
                            nc.vector.tensor_add(hslice, g[:], d[:])
                        # next step's table chunk for these nodes + AG firing
                        if not last:
                            if not skip_table:
                                build_table_chunk(in_nxt, wdx)
                            if (wdx + 1) % wins_per_half == 0:
                                fire_ag(in_nxt, out_nxt, wdx // wins_per_half)

                    for wdx in range(NWIN):
                        psA, Atsb = emit_window_front(wdx)
                        if pend is not None:
                            emit_gru(*pend)
                        aTs = emit_window_back(wdx, psA, Atsb)
                        pend = (wdx, aTs)
                    emit_gru(*pend)
                    if not last:
                        in_cc, out_cc = in_nxt, out_nxt

            # ---------------- readout ----------------
            with tc.tile_pool(name="rsb", bufs=2) as rsb, \
                 tc.tile_pool(name="rx", bufs=1) as rxp, \
                 tc.tile_pool(name="pR", bufs=1, space="PSUM") as pR, \
                 tc.tile_pool(name="pV", bufs=1, space="PSUM") as pV:
                res_sb = pp.tile([1, GPC], f32)
                xTb = rxp.tile([128, NPC], bf)
                nc.sync.dma_start(xTb[:], xT_d.ap())

                def pool_step(y, width, ksz, tag):
                    # maxpool stride 2 over free dim, VALID
                    outw = (width - ksz) // 2 + 1
                    pout = rsb.tile([128, outw], bf, tag=tag)
                    ab = y[:, 0:2 * outw].rearrange("p (n t) -> p n t", t=2)
                    a0 = ab[:, :, 0]
                    a1 = ab[:, :, 1]
                    if ksz == 3:
                        tmp = rsb.tile([128, outw], bf, tag=tag + "_t")
                        nc.vector.tensor_max(tmp[:], a0, a1)
                        a2 = y[:, 2:2 * outw + 2].rearrange("p (n t) -> p n t", t=2)[:, :, 0]
                        nc.vector.tensor_max(pout[:], tmp[:], a2)
                    else:
                        nc.vector.tensor_max(pout[:], a0, a1)
                    return pout

                for gidx in range(GPC):
                    g0 = gidx * LG
                    # ---- Y path (h only, 256 ch) ----
                    y2p = []
                    for mo in range(2):
                        psY = pR.tile([128, 254], f32, space="PSUM", tag="psY")
                        n_mm = 0
                        for tap in range(3):
                            for kc in range(2):
                                nc.tensor.matmul(
                                    psY[:],
                                    lhsT=W(f"c1_{tap}_{kc}_{mo}"),
                                    rhs=hT[kc][:, g0 + tap:g0 + tap + 254],
                                    start=(n_mm == 0), stop=(n_mm == 5))
                                n_mm += 1
                        y1 = rsb.tile([128, 254], bf, tag=f"y1_{mo}")
                        nc.scalar.activation(y1[:], psY[:], AF.Relu,
                                             bias=Bias("c1b", mo))
                        y2p.append(pool_step(y1, 254, 3, f"p3_{mo}"))
                    y3p = []
                    for mo in range(2):
                        psY2 = pR.tile([128, 126], f32, space="PSUM", tag="psY2")
                        for kc in range(2):
                            nc.tensor.matmul(psY2[:], lhsT=W(f"c2_{kc}_{mo}"),
                                             rhs=y2p[kc][:],
                                             start=(kc == 0), stop=(kc == 1))
                        y2 = rsb.tile([128, 126], bf, tag=f"y2_{mo}")
                        nc.scalar.activation(y2[:], psY2[:], AF.Relu,
                                             bias=Bias("c2b", mo))
                        y3p.append(pool_step(y2, 126, 2, f"yp_{mo}"))
                    psy = pV.tile([1, 63], f32, space="PSUM", tag="psy")
                    for kc in range(2):
                        nc.tensor.matmul(psy[:], lhsT=W(f"wy_{kc}"),
                                         rhs=y3p[kc][:],
                                         start=(kc == 0), stop=(kc == 1))
                    # ---- Z path (concat h|x, 384 ch) ----
                    cch = [hT[0], hT[1], xTb]
                    z2p = []
                    for mo in range(3):
                        psZ = pR.tile([128, 254], f32, space="PSUM", tag="psZ")
                        n_mm = 0
                        for tap in range(3):
                            for kc in range(3):
                                nc.tensor.matmul(
                                    psZ[:],
                                    lhsT=W(f"cc1_{tap}_{kc}_{mo}"),
                                    rhs=cch[kc][:, g0 + tap:g0 + tap + 254],
                                    start=(n_mm == 0), stop=(n_mm == 8))
                                n_mm += 1
                        z1 = rsb.tile([128, 254], bf, tag=f"z1_{mo}")
                        nc.scalar.activation(z1[:], psZ[:], AF.Relu,
                                             bias=Bias("cc1b", mo))
                        z2p.append(pool_step(z1, 254, 3, f"zp_{mo}"))
                    z3p = []
                    for mo in range(3):
                        psZ2 = pR.tile([128, 126], f32, space="PSUM", tag="psZ2")
                        for kc in range(3):
                            nc.tensor.matmul(psZ2[:], lhsT=W(f"cc2_{kc}_{mo}"),
                                             rhs=z2p[kc][:],
                                             start=(kc == 0), stop=(kc == 2))
                        z2 = rsb.tile([128, 126], bf, tag=f"z2_{mo}")
                        nc.scalar.activation(z2[:], psZ2[:], AF.Relu,
                                             bias=Bias("cc2b", mo))
                        z3p.append(pool_step(z2, 126, 2, f"zq_{mo}"))
                    psz = pV.tile([1, 63], f32, space="PSUM", tag="psz")
                    for kc in range(3):
                        nc.tensor.matmul(psz[:], lhsT=W(f"wz_{kc}"),
                                         rhs=z3p[kc][:],
                                         start=(kc == 0), stop=(kc == 2))
                    # ---- combine ----
                    ty = rsb.tile([1, 63], f32, tag="ty")
                    tz = rsb.tile([1, 63], f32, tag="tz")
                    pr = rsb.tile([1, 63], f32, tag="pr")
                    sm = rsb.tile([1, 1], f32, tag="sm")
                    nc.vector.tensor_scalar_add(ty[:], psy[:], Bias1("by"))
                    nc.vector.tensor_scalar_add(tz[:], psz[:], Bias1("bz_"))
                    nc.vector.tensor_mul(pr[:], ty[:], tz[:])
                    nc.vector.tensor_reduce(sm[:], pr[:],
                                            axis=mybir.AxisListType.X, op=ALU.add)
                    nc.scalar.activation(res_sb[0:1, gidx:gidx + 1], sm[:],
                                         AF.Sigmoid, scale=1.0 / 63.0)
                nc.sync.dma_start(out_d.ap(), res_sb[0:1, :])
    nc.finalize()
    return nc


# ---------------------------------------------------------------------------
# host entry
# ---------------------------------------------------------------------------
def _prepare(inputs):
    features = np.asarray(inputs["features"], np.float32)
    src = np.asarray(inputs["src"]).astype(np.int64)
    dst = np.asarray(inputs["dst"]).astype(np.int64)
    etype = np.asarray(inputs["etype"]).astype(np.int64)
    wl = _make_wlayout()
    bl = _make_blayout()
    Wimg = _pack_weights(
        wl,
        np.asarray(inputs["W_msg"], np.float32),
        np.asarray(inputs["gru_Wi"], np.float32),
        np.asarray(inputs["gru_Wh"], np.float32),
        np.asarray(inputs["conv1_w"], np.float32),
        np.asarray(inputs["conv2_w"], np.float32),
        np.asarray(inputs["convc1_w"], np.float32),
        np.asarray(inputs["convc2_w"], np.float32),
        np.asarray(inputs["wy"], np.float32),
        np.asarray(inputs["wz"], np.float32),
    )
    Bimg = _pack_biases(
        bl,
        np.asarray(inputs["gru_bi"], np.float32),
        np.asarray(inputs["gru_bh"], np.float32),
        np.asarray(inputs["conv1_b"], np.float32),
        np.asarray(inputs["conv2_b"], np.float32),
        np.asarray(inputs["convc1_b"], np.float32),
        np.asarray(inputs["convc2_b"], np.float32),
        np.asarray(inputs["by"], np.float32),
        np.asarray(inputs["bz"], np.float32),
    )
    b_msg = np.asarray(inputs["b_msg"], np.float32)
    assert np.abs(b_msg).max() == 0.0, "nonzero b_msg not supported"

    idx16, S_img, CB = _preprocess_edges(src, dst, etype)

    in_maps = []
    for c in range(NCORES):
        feats = features[c * NPC:(c + 1) * NPC]  # [4096, 128]
        xT = feats.T.astype(BF16)                # [128, 4096]
        h0T = np.zeros((OUT, NPC), np.float32)
        h0T[:IN_DIM] = feats.T
        im = {
            "h0T": h0T.astype(BF16),
            "xT": xT,
            "Wimg": Wimg,
            "Bimg": Bimg,
            "Simg": S_img[c],
            "idx": idx16[c],
        }
        in_maps.append(im)
    return wl, bl, CB, in_maps


def kernel(**inputs):
    wl, bl, CB, in_maps = _prepare(inputs)
    nc = build_program(CB, wl, bl)
    res = run_bass_kernel_spmd(nc, in_maps, core_ids=list(range(NCORES)))
    out = np.concatenate([res.results[c]["out"] for c in range(NCORES)])
    return out.astype(np.float32)
